# revision 8
# baseline (speedup 1.0000x reference)
"""Weighted-BCE loss kernel for Trainium2 (8 NeuronCores, SPMD data-parallel).

Reference math (torch-style BCELoss with class-balancing weights):
    n   = len(x), s = sum(gt)
    w0  = n / (2*(n-s)),  w1 = n / (2*s)
    L1  = max(log(x),     -100)
    L0  = max(log1p(-x),  -100)
    loss = mean( where(gt==0, w0, w1) * -(gt*L1 + (1-gt)*L0) )

Only ONE of log(x) / log(1-x) is needed per element (selected by gt), so
instead of two Ln passes we compute the selected operand in one shot:
    z = gt ? x : 1-x  =  1 - |x - gt|          (gt in {0,1})
Global sums computed shard-locally (weights only need the GLOBAL s):
    A = sum(gt * Lz)   [DVE STT accum]  = sum_{gt=1} log x
    T = sum(Lz)        [ACT accum, free on the Ln pass]
    S = sum(gt)        [ACT Copy accum]
    loss = -( A/(2S) + (T-A)/(2(n-S)) )

Intermediates (w, d=|w|, Lz, gt-copy) are bf16: DVE runs 16-bit ops at
2 elem/cycle/lane, halving the abs- and A-pass cost; rel-err impact is
~0.3% (tolerance 2e-2).  Since bf16 |w| can round to exactly 1.0, the
Ln pass uses bias 1+2^-17 so its input stays >= 2^-17 (no -inf; the
affected elements are the ~0.1% with true z < 2^-10, error ~2.5e-3).

Per [128, 4096] tile the DMA needs ~12us; engine budgets stay under:
    DVE  w-STT (f32 in) 4.4 + abs bf16 2.2 + A bf16 2.2   ~9us
    ACT  Ln 3.6 + Copy(gt) 3.6 + gt-DMA issue 0.7         ~8um
    SP   x DMA
Overlap details: the A-STT consumes the bf16 gt-copy (junk_s), not gt
itself, so gt buffers free right after w/Copy and the gt DMA queue
never waits on the tile's full dependency chain; the A-STT is emitted
one tile late so DVE always has independent w/abs work queued ahead of
the cross-engine Ln dependency.  Descending tile sizes shrink the
pipeline-drain tail.  Host gathers [128, 3*NT] partials from all 8
cores and finishes the (tiny) all-reduce + scalar math in float64.
"""

import numpy as np
from contextlib import ExitStack

import concourse.bass as bass
import concourse.bacc as bacc
import concourse.mybir as mybir
import concourse.tile as tile
from concourse.alu_op_type import AluOpType
from concourse.bass_utils import run_bass_kernel_spmd

N_TOTAL = 16777216
N_CORES = 8
PER_CORE = N_TOTAL // N_CORES   # 2097152
P = 128
FD = PER_CORE // P              # 16384 free elements per partition
TILE_SIZES = [4096, 4096, 4096, 2048, 1024, 512, 512]
assert sum(TILE_SIZES) == FD
NT = len(TILE_SIZES)
X_CLAMP = 5.9604645e-08         # 2^-24, fixes x==0 (w would hit -1 in f32)
LN_BIAS = 1.0 + 2.0**-17        # keeps Ln input positive even when d==1.0
LOG_CLAMP = -100.0

# Optional instrumentation knobs for a driver script (harness never sets them).
TRACE = False
LAST_RESULTS = None

_NC_CACHE = None


def _build():
    f32 = mybir.dt.float32
    bf16 = mybir.dt.bfloat16
    i32 = mybir.dt.int32
    Ln = mybir.ActivationFunctionType.Ln

    nc = bacc.Bacc("TRN2")
    x_in = nc.declare_dram_parameter("x", [P, FD], f32, isOutput=False)
    g_in = nc.declare_dram_parameter("gt", [P, FD], i32, isOutput=False)
    # one packed output: columns [A | T | S], NT each
    out_all = nc.declare_dram_parameter("out_all", [P, 3 * NT], f32, isOutput=True)

    # custom Ln bias constant (only 0.0/1.0 are pre-registered const APs)
    bias_t = nc.alloc_sbuf_tensor("ln_bias", [P, 1], f32)
    nc.gpsimd.memset(bias_t.ap(), LN_BIAS)
    nc.all_engine_barrier()

    with tile.TileContext(nc) as tc, ExitStack() as ctx:
        xp = ctx.enter_context(tc.tile_pool(name="xp", bufs=3))
        gp = ctx.enter_context(tc.tile_pool(name="gp", bufs=3))
        wp = ctx.enter_context(tc.tile_pool(name="wp", bufs=2))
        dp = ctx.enter_context(tc.tile_pool(name="dp", bufs=2))
        lp = ctx.enter_context(tc.tile_pool(name="lp", bufs=2))
        jsp = ctx.enter_context(tc.tile_pool(name="jsp", bufs=3))
        jp = ctx.enter_context(tc.tile_pool(name="jp", bufs=1))
        accp = ctx.enter_context(tc.tile_pool(name="accp", bufs=1))

        accA = accp.tile([P, NT], f32)
        accT = accp.tile([P, NT], f32)
        accS = accp.tile([P, NT], f32)
        groups = [accA, accT, accS]

        def col(group, i):
            return groups[group][:, i : i + 1]

        def emit_A(i, lz, junk_s, tfd):
            junk_a = jp.tile([P, tfd], bf16, tag="junk_a")
            nc.vector.scalar_tensor_tensor(
                junk_a[:], lz[:], LOG_CLAMP, junk_s[:],
                AluOpType.max, AluOpType.mult,
                accum_out=col(0, i),
            )

        pending_A = None  # (i, lz, junk_s, tfd) emitted one tile late
        off = 0
        for i, tfd in enumerate(TILE_SIZES):
            sl = slice(off, off + tfd)
            off += tfd
            xt = xp.tile([P, tfd], f32, tag="xt")
            gt_t = gp.tile([P, tfd], i32, tag="gt")
            # two HWDGE queues: x via SP(sync), gt via the ACT sequencer
            nc.sync.dma_start(xt[:], x_in[:, sl])
            nc.scalar.dma_start(gt_t[:], g_in[:, sl])

            # w = max(x, 2^-24) - gt  in [-1, 1], bf16
            wt = wp.tile([P, tfd], bf16, tag="w")
            nc.vector.scalar_tensor_tensor(
                wt[:], xt[:], X_CLAMP, gt_t[:],
                AluOpType.max, AluOpType.subtract,
            )
            # d = |w| = max(-w, w), bf16 (2 elem/cycle on DVE)
            dt_ = dp.tile([P, tfd], bf16, tag="d")
            nc.vector.scalar_tensor_tensor(
                dt_[:], wt[:], -1.0, wt[:],
                AluOpType.mult, AluOpType.max,
            )
            # S = sum(gt) via ACT Copy (i32 -> bf16, exact for 0/1)
            junk_s = jsp.tile([P, tfd], bf16, tag="junk_s")
            nc.scalar.activation(
                junk_s[:], gt_t[:], mybir.ActivationFunctionType.Copy,
                accum_out=col(2, i),
            )
            # Lz = Ln(1 + 2^-17 - d) = log(gt ? x : 1-x), accum -> T
            lz = lp.tile([P, tfd], bf16, tag="lz")
            nc.scalar.activation(
                lz[:], dt_[:], Ln, bias=bias_t.ap(), scale=-1.0,
                accum_out=col(1, i),
            )
            # A-STT for the PREVIOUS tile (keeps independent DVE work ahead
            # of the cross-engine Ln dependency)
            if pending_A is not None:
                emit_A(*pending_A)
            pending_A = (i, lz, junk_s, tfd)

        emit_A(*pending_A)

        for k, g in enumerate(groups):
            nc.sync.dma_start(out_all[:, k * NT : (k + 1) * NT], g[:])

    nc.compile()
    return nc


def get_nc():
    global _NC_CACHE
    if _NC_CACHE is None:
        _NC_CACHE = _build()
    return _NC_CACHE


def make_in_maps(x, gt):
    x = np.ascontiguousarray(np.asarray(x, dtype=np.float32).reshape(-1))
    gt = np.ascontiguousarray(np.asarray(gt, dtype=np.int32).reshape(-1))
    assert x.shape == (N_TOTAL,) and gt.shape == (N_TOTAL,)
    in_maps = []
    for c in range(N_CORES):
        sl = slice(c * PER_CORE, (c + 1) * PER_CORE)
        in_maps.append({
            "x": x[sl].reshape(P, FD),
            "gt": gt[sl].reshape(P, FD),
        })
    return in_maps


def combine(results):
    """All-reduce the per-core partial sums and finish the loss formula."""
    A = T = S = 0.0
    for r in results:
        o = r["out_all"].astype(np.float64)
        A += o[:, 0 * NT : 1 * NT].sum()
        T += o[:, 1 * NT : 2 * NT].sum()
        S += o[:, 2 * NT : 3 * NT].sum()
    n = float(N_TOTAL)
    result = -(A / (2.0 * S) + (T - A) / (2.0 * (n - S)))
    return np.array(result, dtype=np.float32)


def kernel(x, gt):
    global LAST_RESULTS
    nc = get_nc()
    in_maps = make_in_maps(x, gt)
    br = run_bass_kernel_spmd(nc, in_maps, list(range(N_CORES)))
    LAST_RESULTS = br
    return combine(br.results)
